# revision 45
# baseline (speedup 1.0000x reference)
"""Trainium2 Bass kernel for nn_Attention_4501125726440 (sparse_attention).

Full attention layer: QKV projections, per-head-dim RMSNorm on Q/K, full-head
RoPE, causal attention with sink-augmented softmax, output projection.

Sharding: 8 cores = (batch b in {0,1}) x (head-group hg in {0..3}, 4 heads
each).  Each core computes its batch's 4 heads end-to-end plus the partial
output projection through the matching 256 rows of wo^T; the host sums the 4
partials per batch (row-parallel tensor parallelism).

Device layout (B=2, S=2048, D=1024, H=16, HD=64; per core: 4 heads):
  - All HBM inputs are host-relaid so each SBUF partition's data is one
    contiguous HBM run (16-32KB descriptors).
  - The PE has a p-state ramp (max clock only after ~3us of gapless
    execution), so the kernel is organized around keeping the PE queue free
    of cross-engine dependencies: projections run d-outer in long bursts;
    the RMSNorm sum-of-squares matmuls of each unit are deferred behind the
    NEXT unit's matmul block; attention's score->exp->PV loop is software
    pipelined depth 2; out-projection for q-range t is emitted inside range
    t+1's kb loop.
  - Q/K are computed feature-major ([256 feats on partitions, S free]); the
    norm weight (and HD**-0.25 of the softmax scale) is folded into wq/wk on
    the host, and the sum-of-squares reduction matrix carries 1/w^2 so the
    RMS comes out right.  The reciprocal RMS is broadcast across partitions
    by GpSimd (partition_broadcast is its only ucode library in use --
    mixing libraries forces slow LIBRARY_RELOADs).
  - RoPE rotate-half: the sign lives in the host sin table, so the device
    does plain partition-swap copies; all elementwise work runs on full-S
    (2048-wide) bf16 SBUF tiles where the DVE hits its 2x/4x modes.
  - Attention computes transposed score blocks sT[k,q] = K-block^T @ Q so exp
    applies block-wise and the P@V matmul consumes them directly (lhsT = V
    block).  V carries an all-ones 65th column, so row 64 of the PV psum
    accumulator is the softmax denominator for free.  No max-subtraction
    needed: post-RMSNorm |q|,|k| <= 8*max|w| keeps |scores| <= ~8.

Matmul inputs are bf16 (cast on host); all accumulation is fp32 in PSUM.
"""

import sys

import ml_dtypes
import numpy as np

_REPO = "/opt/trn_rl_repo"
if _REPO not in sys.path:
    sys.path.insert(0, _REPO)

import concourse.bacc as bacc  # noqa: E402
import concourse.mybir as mybir  # noqa: E402
import concourse.tile as tile  # noqa: E402
from concourse.bass_utils import run_bass_kernel_spmd  # noqa: E402

B, S, D = 2, 2048, 1024
H = 16
HD = 64
HEADS_PER_CORE = 4
FEATS = HEADS_PER_CORE * HD  # 256
EPS = 1e-6
ROPE_BASE = 10000.0
N_CORES = 8

F32 = mybir.dt.float32
BF16 = mybir.dt.bfloat16
BF16_NP = ml_dtypes.bfloat16

DCH = D // 128      # 8 contraction chunks for projections
FCH = FEATS // 128  # 2 feature chunks (2 heads each)
SQ = 512            # q-tile width in attention / projection free chunks
NSQ = S // SQ       # 4
NKB = S // 128      # 16 key blocks
NQT = S // 128      # 16 128-row q tiles

EXP = mybir.ActivationFunctionType.Exp
SQRT = mybir.ActivationFunctionType.Sqrt


def build_program():
    nc = bacc.Bacc("TRN2", target_bir_lowering=False, debug=False)

    xTr = nc.dram_tensor("xTr", [128, DCH * S], BF16, kind="ExternalInput").ap()
    wqTr = nc.dram_tensor("wqTr", [128, DCH * FEATS], BF16, kind="ExternalInput").ap()
    wkTr = nc.dram_tensor("wkTr", [128, DCH * FEATS], BF16, kind="ExternalInput").ap()
    wvTr = nc.dram_tensor("wvTr", [128, DCH * FEATS], BF16, kind="ExternalInput").ap()
    woTr = nc.dram_tensor("woTr", [128, FCH * D], BF16, kind="ExternalInput").ap()
    cosT = nc.dram_tensor("cosT", [128, S], BF16, kind="ExternalInput").ap()
    sinT = nc.dram_tensor("sinT", [128, S], BF16, kind="ExternalInput").ap()
    trimask = nc.dram_tensor("trimask", [128, 128], BF16, kind="ExternalInput").ap()
    qss = nc.dram_tensor("qss", [128, HD + 1], BF16, kind="ExternalInput").ap()
    kss = nc.dram_tensor("kss", [128, HD + 1], BF16, kind="ExternalInput").ap()
    sinkexp = nc.dram_tensor("sinkexp", [HD + 1, FCH], F32, kind="ExternalInput").ap()
    y = nc.dram_tensor("y", [S, D], F32, kind="ExternalOutput").ap()

    xTr3 = xTr.rearrange("p (o s) -> p o s", o=DCH)      # [128, 8, S]
    wqTr3 = wqTr.rearrange("p (o f) -> p o f", o=DCH)    # [128, 8, 256]
    wkTr3 = wkTr.rearrange("p (o f) -> p o f", o=DCH)
    wvTr3 = wvTr.rearrange("p (o f) -> p o f", o=DCH)
    woTr3 = woTr.rearrange("p (c d) -> p c d", c=FCH)    # [128, 2, 1024]

    with tile.TileContext(nc) as tc, nc.allow_low_precision(reason="bf16 matmul pipeline"):
        with (
            tc.tile_pool(name="persist", bufs=1) as persist,
            tc.tile_pool(name="consts", bufs=1) as consts,
        ):
            # Persistent SBUF tensors
            q_sb = persist.tile([128, FCH, S], BF16, tag="q_sb")
            k_sb = persist.tile([128, FCH, S], BF16, tag="k_sb")
            # V: [kpos, kb, head, HD+1]; the 65th column is all-ones so the
            # PV psum row HD accumulates the softmax denominator.
            v_sb = persist.tile([128, NKB, HEADS_PER_CORE, HD + 1], BF16, tag="v_sb")
            ot_sb = persist.tile([128, FCH, S], BF16, tag="ot_sb")

            cos_sb = consts.tile([128, S], BF16, tag="cos_sb")
            sin_sb = consts.tile([128, S], BF16, tag="sin_sb")
            mask_sb = consts.tile([128, 128], BF16, tag="mask_sb")
            qss_sb = consts.tile([128, HD + 1], BF16, tag="qss_sb")
            kss_sb = consts.tile([128, HD + 1], BF16, tag="kss_sb")
            sink_sb = consts.tile([HD + 1, FCH], F32, tag="sink_sb")
            eps_sb = consts.tile([128, 1], F32, tag="eps_sb")

            nc.gpsimd.dma_start(cos_sb[:], cosT)
            nc.gpsimd.dma_start(sin_sb[:], sinT)
            nc.gpsimd.dma_start(mask_sb[:], trimask)
            nc.gpsimd.dma_start(qss_sb[:], qss)
            nc.gpsimd.dma_start(kss_sb[:], kss)
            nc.gpsimd.dma_start(sink_sb[:], sinkexp)
            nc.vector.memset(eps_sb[:], EPS)
            nc.vector.memset(v_sb[:, :, :, HD:HD + 1], 1.0)

            # PE warm-up: dense junk matmuls during the input DMA ramp so
            # the p-state reaches 2.4 GHz before real work arrives.
            with tc.tile_pool(name="warm", bufs=1) as warm, \
                 tc.tile_pool(name="warm_ps", bufs=1, space="PSUM") as warm_ps:
                wz = warm.tile([128, SQ], BF16, tag="wz")
                nc.vector.memset(wz[:], 0.0)
                wps = warm_ps.tile([128, SQ], F32, tag="wps")
                for i in range(76):
                    nc.tensor.matmul(wps[:], wz[:, 0:128], wz[:],
                                     start=True, stop=True)

            # ---------------- Phase 1: QKV projections -----------------
            with (
                tc.tile_pool(name="p1", bufs=2) as p1,
                tc.tile_pool(name="p1x", bufs=1) as p1x,
                tc.tile_pool(name="p1work", bufs=3) as p1w,
                tc.tile_pool(name="p1ps", bufs=6, space="PSUM") as p1ps,
                tc.tile_pool(name="p1ps_small", bufs=2, space="PSUM") as p1pss,
            ):
                xt_sb = p1x.tile([128, DCH, S], BF16, tag="xt_sb")
                wk_sb = p1.tile([128, DCH, FEATS], BF16, tag="w_sb", name="wk_sb",
                                bufs=3)
                nc.sync.dma_start(wk_sb[:], wkTr3)
                wq_sb = p1.tile([128, DCH, FEATS], BF16, tag="w_sb", name="wq_sb",
                                bufs=3)
                nc.scalar.dma_start(wq_sb[:], wqTr3)
                wv_sb = p1.tile([128, DCH, FEATS], BF16, tag="w_sb", name="wv_sb",
                                bufs=3)
                nc.sync.dma_start(wv_sb[:], wvTr3)
                for i, eng in enumerate((nc.sync, nc.scalar, nc.gpsimd, nc.sync)):
                    eng.dma_start(xt_sb[:, 2 * i:2 * i + 2, :],
                                  xTr3[:, 2 * i:2 * i + 2, :])

                def qk_unit(w_sb, f, ss_w_sb, dst_sb):
                    """Emit the 32 projection matmuls + psum->sbuf copies.

                    Returns a finisher that emits the sum-of-squares matmuls
                    (the only PE work in the norm/rope chain) plus the
                    scalar/vector/gpsimd tail; call it after the NEXT unit's
                    matmul block so the PE never stalls on the chain.
                    """
                    pss = []
                    raw = p1w.tile([128, S], BF16, tag="raw")
                    sq = p1w.tile([128, S], BF16, tag="sq")
                    for s in range(NSQ):
                        pss.append(p1ps.tile([128, SQ], F32, tag="qkv_ps", name="qkv_ps"))
                    for d in range(DCH):
                        for s in range(NSQ):
                            nc.tensor.matmul(
                                pss[s][:],
                                w_sb[:, d, f * 128:(f + 1) * 128],
                                xt_sb[:, d, s * SQ:(s + 1) * SQ],
                                start=(d == 0),
                                stop=(d == DCH - 1),
                            )
                    for s in range(NSQ):
                        nc.scalar.copy(raw[:, s * SQ:(s + 1) * SQ], pss[s][:])
                        nc.scalar.square(sq[:, s * SQ:(s + 1) * SQ], pss[s][:])

                    def ss_stage():
                        # head 0's sum lands on partition 0, head 1's on
                        # partition 64 (partition offsets must be 32-aligned
                        # for the single-row broadcasts below).
                        rms = p1w.tile([HD + 1, S], F32, tag="rms")
                        for s in range(NSQ):
                            ss_ps = p1pss.tile([128, SQ], F32, tag="aux_ps",
                                               name="ss_ps")[0:HD + 1]
                            nc.tensor.matmul(ss_ps[:], ss_w_sb[:],
                                             sq[:, s * SQ:(s + 1) * SQ],
                                             start=True, stop=True)
                            nc.scalar.activation(
                                rms[:, s * SQ:(s + 1) * SQ], ss_ps[:], SQRT,
                                bias=eps_sb[0:HD + 1, :], scale=1.0 / HD,
                            )
                        return rms

                    def finish(rms):
                        rinv = rms
                        nc.vector.reciprocal_approx_fast(rinv[:], rms[:])
                        # broadcast 1/rms across the 64 head partitions on
                        # GpSimd (full 128 per head; the qn muls then read
                        # partition-matched halves).  partition_broadcast
                        # inputs must start at partition 0, hence the split
                        # single-row casts.
                        rbs = []
                        for hh in range(2):
                            rmsb = p1w.tile([1, S], BF16, tag="rmsb",
                                            name=f"rmsb{hh}", bufs=4)
                            nc.vector.tensor_copy(
                                rmsb[:], rinv[hh * HD:hh * HD + 1, :])
                            rb = p1w.tile([128, S], BF16, tag="rb",
                                          name=f"rb{hh}")
                            nc.gpsimd.partition_broadcast(rb[:], rmsb[:])
                            rbs.append(rb)
                        qn = p1w.tile([128, S], BF16, tag="qn")
                        nc.vector.tensor_mul(qn[0:HD, :], raw[0:HD, :],
                                             rbs[0][0:HD, :])
                        nc.vector.tensor_mul(qn[HD:128, :], raw[HD:128, :],
                                             rbs[1][HD:128, :])
                        # RoPE: dst = qn*cos + rot(qn*sin'); the rotate-half
                        # sign lives in sin', so rot is plain swap copies.
                        qs = p1w.tile([128, S], BF16, tag="qsqc", bufs=4,
                                      name="qs")
                        nc.vector.tensor_mul(qs[:], qn[:], sin_sb[:])
                        rot = p1w.tile([128, S], BF16, tag="rot")
                        for base in (0, 64):
                            nc.vector.tensor_copy(rot[base:base + 32, :],
                                                  qs[base + 32:base + 64, :])
                            nc.vector.tensor_copy(rot[base + 32:base + 64, :],
                                                  qs[base:base + 32, :])
                        qc = p1w.tile([128, S], BF16, tag="qsqc", bufs=4,
                                      name="qc")
                        nc.vector.tensor_mul(qc[:], qn[:], cos_sb[:])
                        nc.vector.tensor_add(dst_sb[:, f, :], qc[:], rot[:])
                    return ss_stage, finish

                # K before Q: attention needs k_sb earliest.  Each unit's
                # ss matmuls go right after the NEXT unit's block (their sq
                # input is ready, ~1us PE wait at most) so the vector-bound
                # norm/rope chains start as early as possible.
                ss_k0, fin_k0 = qk_unit(wk_sb, 0, kss_sb, k_sb)
                ss_q0, fin_q0 = qk_unit(wq_sb, 0, qss_sb, q_sb)
                rms_k0 = ss_k0()
                ss_k1, fin_k1 = qk_unit(wk_sb, 1, kss_sb, k_sb)
                rms_q0 = ss_q0()
                fin_k0(rms_k0)
                ss_q1, fin_q1 = qk_unit(wq_sb, 1, qss_sb, q_sb)
                rms_k1 = ss_k1()
                fin_q0(rms_q0)
                rms_q1 = ss_q1()
                fin_k1(rms_k1)

                # V projection: sequence-major, lhsT = xT tile.
                for g in range(NQT // 4):
                    pss = []
                    for qi in range(4):
                        pss.append(p1ps.tile([128, SQ], F32, tag="qkv_ps", name="v_ps"))
                    for d in range(DCH):
                        for qi in range(4):
                            qt = g * 4 + qi
                            nc.tensor.matmul(
                                pss[qi][:, :FEATS],
                                xt_sb[:, d, qt * 128:(qt + 1) * 128],
                                wv_sb[:, d, :],
                                start=(d == 0),
                                stop=(d == DCH - 1),
                            )
                    for qi in range(4):
                        qt = g * 4 + qi
                        nc.scalar.copy(
                            v_sb[:, qt, :, 0:HD],
                            pss[qi][:, 0:FEATS].rearrange(
                                "p (h e) -> p h e", h=HEADS_PER_CORE))
                    if g == 0:
                        fin_q1(rms_q1)

            # ------------- Phase 2+3: attention + output projection ------
            with (
                tc.tile_pool(name="p2p", bufs=6) as p2p,
                tc.tile_pool(name="p2w", bufs=2) as p2w,
                tc.tile_pool(name="p3", bufs=1) as p3,
                tc.tile_pool(name="p3w", bufs=2) as p3w,
                tc.tile_pool(name="p2s_ps", bufs=2, space="PSUM") as p2sps,
                tc.tile_pool(name="p2o_ps", bufs=2, space="PSUM") as p2ops,
                tc.tile_pool(name="p3ps", bufs=2, space="PSUM") as p3ps,
            ):
                wo_sb = p3.tile([128, FCH, D], BF16, tag="wo_sb")
                nc.sync.dma_start(wo_sb[:], woTr3)

                def make_outproj(t, use_scalar=False):
                    def emit():
                        for qi in range(SQ // 128):
                            qt = t * (SQ // 128) + qi
                            y_sb = p3w.tile([128, D], F32, tag="y_sb")
                            for n in range(D // SQ):
                                y_ps = p3ps.tile([128, SQ], F32, tag="y_ps")
                                for c in range(FCH):
                                    nc.tensor.matmul(
                                        y_ps[:],
                                        ot_sb[:, c, qt * 128:(qt + 1) * 128],
                                        wo_sb[:, c, n * SQ:(n + 1) * SQ],
                                        start=(c == 0),
                                        stop=(c == FCH - 1),
                                    )
                                if use_scalar:
                                    nc.scalar.copy(
                                        y_sb[:, n * SQ:(n + 1) * SQ], y_ps[:])
                                else:
                                    nc.vector.tensor_copy(
                                        y_sb[:, n * SQ:(n + 1) * SQ], y_ps[:])
                            nc.sync.dma_start(y[qt * 128:(qt + 1) * 128, :],
                                              y_sb[:])
                    return emit

                pending = None
                # custom unit order: the first two units need only the f=0
                # chunks, giving fin_q1's vector chain time to drain; each
                # outproj(t) still lands one unit after its (t,1).
                UNITS = [(3, 0), (2, 0), (3, 1), (2, 1),
                         (1, 0), (0, 0), (1, 1), (0, 1)]
                done_f = {}
                for t, f in UNITS:
                    nkb = (t + 1) * (SQ // 128)  # causal: key blocks 0..nkb-1
                    if True:
                        ot_ps = [
                            p2ops.tile([HD + 1, SQ], F32, tag="ot_ps",
                                       name=f"ot_ps{hh}")
                            for hh in range(2)
                        ]
                        psbs = {}

                        def issue_score(kb):
                            j = kb - (t * (SQ // 128))
                            qlo = max(j, 0) * 128
                            sp = p2sps.tile([128, 2 * SQ], F32, tag="s_ps")
                            for hh in range(2):
                                plo = hh * HD
                                nc.tensor.matmul(
                                    sp[:, hh * SQ + qlo:(hh + 1) * SQ],
                                    k_sb[plo:plo + HD, f,
                                         kb * 128:(kb + 1) * 128],
                                    q_sb[plo:plo + HD, f,
                                         t * SQ + qlo:(t + 1) * SQ],
                                    start=True, stop=True,
                                )
                            pb = p2p.tile([128, 2 * SQ], BF16, tag="p_sb")
                            # full-width exp even for diagonal blocks: the
                            # stale psum columns below qlo are bounded, cheap
                            # to exp, and never read by the PV matmul.
                            nc.scalar.activation(pb[:], sp[:], EXP)
                            if j >= 0:
                                pv3 = pb.rearrange("p (g q) -> p g q", g=2)
                                nc.vector.tensor_mul(
                                    pv3[:, :, qlo:qlo + 128],
                                    pv3[:, :, qlo:qlo + 128],
                                    mask_sb[:, None, :].broadcast_to(
                                        [128, 2, 128]))
                            psbs[kb] = (pb, qlo)

                        def issue_pv(kb):
                            pb, qlo = psbs.pop(kb)
                            for hh in range(2):
                                h = 2 * f + hh
                                nc.tensor.matmul(
                                    ot_ps[hh][:, qlo:],
                                    v_sb[:, kb, h, :],
                                    pb[:, hh * SQ + qlo:(hh + 1) * SQ],
                                    start=(kb == 0),
                                    stop=(kb == nkb - 1),
                                )

                        issue_score(0)
                        issue_score(1)
                        for kb in range(nkb):
                            if kb + 2 < nkb:
                                issue_score(kb + 2)
                            issue_pv(kb)
                            if kb == 3 and pending is not None:
                                pending()
                                pending = None

                        # denominator chain: psum reads + reciprocal on
                        # vector, broadcast on gpsimd, final scale bf16 on
                        # vector -- the PE is never involved.
                        last_unit = (t == 0 and f == 1)
                        # both heads' denominator rows land on partitions 0
                        # and 64 of one tile: a single sink-add + reciprocal
                        # per unit (rows 1-63 hold garbage, never read).
                        den65 = p2w.tile([HD + 1, SQ], F32, tag="den", bufs=4)
                        ot_us = []
                        for hh in range(2):
                            ot_u = p2w.tile([HD, SQ], BF16, tag="ot_u",
                                            name=f"ot_u{hh}", bufs=4)
                            if last_unit:
                                nc.scalar.copy(den65[hh * HD:hh * HD + 1, :],
                                               ot_ps[hh][HD:HD + 1, :])
                                nc.scalar.copy(ot_u[:], ot_ps[hh][0:HD, :])
                            else:
                                nc.vector.tensor_copy(
                                    den65[hh * HD:hh * HD + 1, :],
                                    ot_ps[hh][HD:HD + 1, :])
                                nc.vector.tensor_copy(
                                    ot_u[:], ot_ps[hh][0:HD, :])
                            ot_us.append(ot_u)
                        nc.vector.tensor_scalar_add(
                            den65[:], den65[:], sink_sb[:, f:f + 1])
                        nc.vector.reciprocal_approx_fast(den65[:], den65[:])
                        for hh in range(2):
                            denb = p2w.tile([1, SQ], BF16, tag="denb",
                                            name=f"denb{hh}", bufs=4)
                            nc.vector.tensor_copy(
                                denb[:], den65[hh * HD:hh * HD + 1, :])
                            bc = p2w.tile([HD, SQ], BF16, tag="bc",
                                          name=f"bc{hh}", bufs=4)
                            nc.gpsimd.partition_broadcast(bc[:], denb[:])
                            nc.vector.tensor_mul(
                                ot_sb[hh * HD:(hh + 1) * HD, f,
                                      t * SQ:(t + 1) * SQ],
                                ot_us[hh][:], bc[:])

                    done_f.setdefault(t, set()).add(f)
                    if len(done_f[t]) == FCH:
                        pending = make_outproj(t, use_scalar=(t == 0))
                pending()

    nc.compile()
    return nc


_NC_CACHE = None


def _get_program():
    global _NC_CACHE
    if _NC_CACHE is None:
        _NC_CACHE = build_program()
    return _NC_CACHE


def _b(x):
    return np.ascontiguousarray(np.asarray(x, dtype=np.float32)).astype(BF16_NP)


def _sink65(se):
    """[4] per-core sink exps -> [65, FCH]: row 0 = head 2f, row 64 = 2f+1."""
    out = np.zeros((HD + 1, FCH), dtype=np.float32)
    for f in range(FCH):
        out[0, f] = se[2 * f]
        out[HD, f] = se[2 * f + 1]
    return out


def _chunk_rows(a, nch):
    """[nch*128, N] -> [128, nch*N] so each partition's data is contiguous."""
    n = a.shape[1]
    return np.ascontiguousarray(
        a.reshape(nch, 128, n).transpose(1, 0, 2).reshape(128, nch * n))


def _host_inputs(x, wq, wk, wv, wo, q_norm_w, k_norm_w, sink_logit):
    """Build the 8 per-core input maps."""
    x = np.asarray(x, dtype=np.float32)
    wq = np.asarray(wq, dtype=np.float32)
    wk = np.asarray(wk, dtype=np.float32)
    wv = np.asarray(wv, dtype=np.float32)
    wo = np.asarray(wo, dtype=np.float32)
    q_norm_w = np.asarray(q_norm_w, dtype=np.float32)
    k_norm_w = np.asarray(k_norm_w, dtype=np.float32)
    sink_logit = np.asarray(sink_logit, dtype=np.float32)

    # rope tables, feature-major, duplicated across the two heads per chunk.
    # sin rows 32-63 of each 64-block carry the rotate-half minus sign.
    inv_freq = 1.0 / (ROPE_BASE ** (np.arange(0, HD, 2, dtype=np.float32) / HD))
    tpos = np.arange(S, dtype=np.float32)
    freqs = tpos[:, None] * inv_freq[None, :]           # [S, 32]
    emb = np.concatenate([freqs, freqs], axis=-1)       # [S, 64]
    cosT = _b(np.tile(np.cos(emb).T, (2, 1)))           # [128, S]
    sin_half = np.sin(freqs).T                          # [32, S]
    sinT = _b(np.tile(np.concatenate([sin_half, -sin_half], axis=0), (2, 1)))

    # triangular causal mask for the single diagonal 128-wide band
    kk = np.arange(128)[:, None]
    qq = np.arange(128)[None, :]
    trimask = _b((kk <= qq).astype(np.float32))          # [128, 128]

    # norm weight (and sqrt of softmax scale) folded into wq/wk rows; the
    # sum-of-squares reduction carries 1/(w*sh)^2 to recover plain sum(q^2).
    scale_half = float(HD) ** -0.25
    qw = np.where(np.abs(q_norm_w) < 1e-8, 1e-8, q_norm_w) * scale_half
    kw = np.where(np.abs(k_norm_w) < 1e-8, 1e-8, k_norm_w) * scale_half
    qss = np.zeros((128, HD + 1), dtype=np.float32)
    kss = np.zeros((128, HD + 1), dtype=np.float32)
    for m in range(2):
        qss[m * 64:(m + 1) * 64, m * HD] = 1.0 / qw ** 2
        kss[m * 64:(m + 1) * 64, m * HD] = 1.0 / kw ** 2
    qss = _b(qss)
    kss = _b(kss)
    # per-row fold: output feature o of wq belongs to head-dim o % 64
    qrow = np.tile(qw, D // HD)                          # [1024]
    krow = np.tile(kw, D // HD)

    in_maps = []
    xTr_b = [_chunk_rows(_b(x[b].T), DCH) for b in range(B)]
    for core in range(N_CORES):
        b = core // 4
        hg = core % 4
        rows = slice(hg * FEATS, (hg + 1) * FEATS)
        heads = slice(hg * HEADS_PER_CORE, (hg + 1) * HEADS_PER_CORE)
        in_maps.append({
            "xTr": xTr_b[b],
            "wqTr": _chunk_rows(_b((wq * qrow[:, None])[rows, :].T), DCH),
            "wkTr": _chunk_rows(_b((wk * krow[:, None])[rows, :].T), DCH),
            "wvTr": _chunk_rows(_b(wv[rows, :].T), DCH),
            "woTr": _chunk_rows(_b(wo[:, rows].T), FCH),
            "cosT": cosT,
            "sinT": sinT,
            "trimask": trimask,
            "qss": qss,
            "kss": kss,
            "sinkexp": _sink65(np.exp(sink_logit[heads]).astype(np.float32)),
        })
    return in_maps


def kernel(x, wq, wk, wv, wo, q_norm_w, k_norm_w, sink_logit, _run_kwargs=None):
    nc = _get_program()
    in_maps = _host_inputs(x, wq, wk, wv, wo, q_norm_w, k_norm_w, sink_logit)
    res = run_bass_kernel_spmd(nc, in_maps, core_ids=list(range(N_CORES)),
                               **(_run_kwargs or {}))
    out = np.zeros((B, S, D), dtype=np.float32)
    for core in range(N_CORES):
        out[core // 4] += res.results[core]["y"]
    if _run_kwargs:
        kernel.last_result = res
    return out
